# revision 2
# baseline (speedup 1.0000x reference)
"""Trainium2 Bass kernel for nn_DetectionHead — int16 fixed-point, v3.

Same math as kernel_i16 (see its docstring): host sends
xt = rint(clip(x)*K) - 2*EPSI as int16 (border pad = -2*EPSI), device
computes the hole-excluded 3x3 max + local-max select entirely in exact
integer arithmetic, output as f16 (host divides by K).

v3 layout changes vs v1:
 - Each band's DVE chain is column-split into two independent half-width
   chains (L: image cols [0,1024), R: [1024,2048)) whose ops interleave,
   hiding each op's pipeline-drain behind the other chain's op.
 - No ScalarE shifted copy: odd-element (2-byte) operand offsets measured
   free for int16 TT ops, so q/g/out read v and xps at odd offsets
   directly.
 - xps lives in its own tile (not in-place on xt), so the ScalarE relu
   depends only on the load and runs concurrently with v/c.

Per-band op list (E = EPSI; all tiles int16 except o:f16):
  load xt[P, 6, 2050]
  ScalarE xpsH = relu(xt[:,1:5, 1+h : 1025+h] + E)   h in {0, 1024}
  DVE  vL = max(xt[0:4, 0:1026],    xt[2:6, 0:1026])     -> v[:, 0:1026]
       vR = max(xt[0:4, 1026:2050], xt[2:6, 1026:2050])  -> v[:, 1026:2050]
       cL = max(v[0:1026],    xt[1:5, 0:1026])           -> c[:, 0:1026]
       cR = max(v[1024:2050], xt[1:5, 1024:2050])        -> c[:, 1024:2050]
       m1L = max(c[0:1024], c[2:1026])       in-place -> c[:, 0:1024]
       m1R = max(c[1024:2048], c[1026:2050]) in-place -> c[:, 1024:2048]
       qL  = max(m1L, v[1:1025])             in-place
       qR  = max(m1R, v[1025:2049])          in-place
       gL  = is_lt(qL, xps[0:1024])          in-place
       gR  = is_lt(qR, xps[1024:2048])       in-place
       oL  = mult(xps[0:1024], gL) -> f16
       oR  = mult(xps[1024:2048], gR) -> f16
  store o[P, 4, 2048] f16
"""

import os

import numpy as np

import concourse.bacc as bacc
import concourse.mybir as mybir
import concourse.tile as tile
from concourse import bass_utils
from concourse.ap import AP

CLIP = 6.0
K = float(np.floor(32767.0 / CLIP))      # 5461
EPSI = float(int(np.rint(0.01 * K)))     # 55
B, H, W = 8, 2048, 2048
HP2, WP2 = H + 2, W + 2
P = 128
RB = 4
BAND_H = RB * P           # 512
NBAND = H // BAND_H       # 4
SB = RB + 2
HALF = W // 2             # 1024
I16 = mybir.dt.int16
F16 = mybir.dt.float16
F32 = mybir.dt.float32
MX = mybir.AluOpType.max


def _emit_pipeline(nc, tc, x_d, o_d, out_row_stride, out_offset0, mode="full"):
    do_load = mode in ("full", "dmaonly", "loadonly")
    do_store = mode in ("full", "dmaonly", "storeonly")
    do_compute = mode in ("full", "nodma")
    TT = nc.vector.tensor_tensor
    st = nc.scalar if os.environ.get("STORE_RING") == "scalar" else nc.sync
    with (
        tc.tile_pool(name="iox", bufs=2) as iox,
        tc.tile_pool(name="work", bufs=2) as wp,
        tc.tile_pool(name="consts", bufs=1) as cp,
    ):
        bias = cp.tile([P, 1], F32, tag="bias")
        nc.vector.memset(bias[:], EPSI)
        for t in range(NBAND):
            xt = iox.tile([P, SB, WP2], I16, tag="xt")
            v = wp.tile([P, RB, WP2], I16, tag="v")
            c = wp.tile([P, RB, WP2], I16, tag="c")
            xps = wp.tile([P, RB, W], I16, tag="xps")
            o = wp.tile([P, RB, W], F16, tag="o")
            mid = xt[:, 1 : RB + 1, :]

            if do_load:
                nc.sync.dma_start(
                    out=xt[:],
                    in_=AP(
                        x_d.tensor,
                        t * BAND_H * WP2,
                        [[RB * WP2, P], [WP2, SB], [1, WP2]],
                    ),
                )
            else:
                nc.vector.memset(xt[:], 3.0)

            if do_compute:
                for h in (0, HALF):
                    nc.scalar.activation(
                        out=xps[:, :, h : h + HALF],
                        in_=mid[:, :, 1 + h : 1 + h + HALF],
                        func=mybir.ActivationFunctionType.Relu,
                        bias=bias[:, 0:1],
                    )
                # vL/vR: disjoint writes split at 1026
                TT(out=v[:, :, 0 : HALF + 2],
                   in0=xt[:, 0:RB, 0 : HALF + 2],
                   in1=xt[:, 2:SB, 0 : HALF + 2], op=MX)
                TT(out=v[:, :, HALF + 2 : WP2],
                   in0=xt[:, 0:RB, HALF + 2 : WP2],
                   in1=xt[:, 2:SB, HALF + 2 : WP2], op=MX)
                # cL/cR: disjoint writes split at 1024 (cR reads v tail of L)
                TT(out=c[:, :, 0:HALF + 2],
                   in0=v[:, :, 0:HALF + 2], in1=mid[:, :, 0:HALF + 2], op=MX)
                TT(out=c[:, :, HALF:WP2],
                   in0=v[:, :, HALF:WP2], in1=mid[:, :, HALF:WP2], op=MX)
                # m1 in-place on c
                TT(out=c[:, :, 0:HALF],
                   in0=c[:, :, 0:HALF], in1=c[:, :, 2 : HALF + 2], op=MX)
                TT(out=c[:, :, HALF:W],
                   in0=c[:, :, HALF:W], in1=c[:, :, HALF + 2 : WP2], op=MX)
                # q = max(m1, v center) in-place on c (odd v offsets)
                TT(out=c[:, :, 0:HALF],
                   in0=c[:, :, 0:HALF], in1=v[:, :, 1 : HALF + 1], op=MX)
                TT(out=c[:, :, HALF:W],
                   in0=c[:, :, HALF:W], in1=v[:, :, HALF + 1 : W + 1], op=MX)
                # g = q < xps in-place on c
                for h in (0, HALF):
                    TT(out=c[:, :, h : h + HALF],
                       in0=c[:, :, h : h + HALF],
                       in1=xps[:, :, h : h + HALF], op=mybir.AluOpType.is_lt)
                # out = xps * g -> f16
                for h in (0, HALF):
                    TT(out=o[:, :, h : h + HALF],
                       in0=xps[:, :, h : h + HALF],
                       in1=c[:, :, h : h + HALF], op=mybir.AluOpType.mult)
            else:
                nc.vector.tensor_copy(out=o[:], in_=mid[:, :, 1 : W + 1])

            if do_store:
                st.dma_start(
                    out=AP(
                        o_d.tensor,
                        out_offset0 + t * BAND_H * out_row_stride,
                        [[RB * out_row_stride, P], [out_row_stride, RB], [1, W]],
                    ),
                    in_=o[:],
                )


def _build_program():
    nc = bacc.Bacc(
        "TRN2",
        target_bir_lowering=False,
        debug=False,
        enable_asserts=False,
        num_devices=B,
    )
    x_d = nc.dram_tensor("x", [HP2, WP2], I16, kind="ExternalInput").ap()
    o_d = nc.dram_tensor("out", [H, W], F16, kind="ExternalOutput").ap()
    with tile.TileContext(nc) as tc:
        _emit_pipeline(nc, tc, x_d, o_d, W, 0)
    nc.compile()
    return nc


def _build_timing_program(niter=1, mode="full"):
    nc = bacc.Bacc(
        "TRN2",
        target_bir_lowering=False,
        debug=False,
        enable_asserts=False,
        num_devices=B,
    )
    di = nc.dram_tensor("x", [1, 8], F32, kind="ExternalInput").ap()
    do = nc.dram_tensor("out", [1, 8], F32, kind="ExternalOutput").ap()
    x_d = nc.dram_tensor("xi", [HP2, WP2], I16, kind="Internal").ap()
    o_d = nc.dram_tensor("oi", [HP2, WP2], F16, kind="Internal").ap()
    with tile.TileContext(nc) as tc:
        with tc.tile_pool(name="dummy", bufs=1) as dp:
            dt = dp.tile([1, 8], F32, tag="dummy")
            nc.sync.dma_start(out=dt[:], in_=di[:])
            nc.sync.dma_start(out=do[:], in_=dt[:])
        if niter == 1:
            _emit_pipeline(nc, tc, x_d, o_d, WP2, WP2 + 1, mode)
        else:
            with tc.For_i(0, niter, 1):
                _emit_pipeline(nc, tc, x_d, o_d, WP2, WP2 + 1, mode)
    nc.compile()
    return nc


SIM_TOL = 1.5e-2


def prep_input(x_img: np.ndarray) -> np.ndarray:
    xi = np.clip(np.rint(x_img * np.float32(K)), -32767, 32767)
    xpad = np.full((HP2, WP2), -2.0 * EPSI, dtype=np.float32)
    xpad[1 : H + 1, 1 : W + 1] = xi - np.float32(2.0 * EPSI)
    return np.clip(xpad, -32768, 32767).astype(np.int16)


def postprocess_out(out_raw: np.ndarray) -> np.ndarray:
    return out_raw.astype(np.float32) / np.float32(K)


_NC = None


def _get_program():
    global _NC
    if _NC is None:
        _NC = _build_program()
    return _NC


def kernel(x: np.ndarray) -> np.ndarray:
    x = np.asarray(x, dtype=np.float32)
    assert x.shape == (B, H, W), x.shape
    nc = _get_program()
    in_maps = [{"x": prep_input(x[i])} for i in range(B)]
    res = bass_utils.run_bass_kernel_spmd(nc, in_maps, core_ids=list(range(B)))
    out = np.stack([r["out"] for r in res.results], axis=0)
    return postprocess_out(out)
